# revision 37
# baseline (speedup 1.0000x reference)
"""NTM forward kernel for 8 Trainium2 NeuronCores (data-parallel over batch).

The reference NTM runs a single step from zero state: memory == 0, so the
read vector is exactly zero and the whole addressing path drops out.  What
remains is:
    gates = x @ W_ih.T + (b_ih + b_hh)        (f-gate unused: c0 == 0)
    h     = sigmoid(o) * tanh(sigmoid(i) * tanh(g))
    out   = h @ W_out[:, :H].T + b_out

Precision strategy (tolerance is 2e-2 L2-relative; fp32r everywhere
measures 1.8e-3): the i and o gates sit inside sigmoids whose outputs are
then attenuated through the h product, so ~3.7% gate error contributes
only ~1.4% to the output.  They run as fp8-e4m3 DoubleRow matmuls (K=256
per instruction, 2x fp32r throughput, 215 ns vs 229 ns per instruction
measured for half the work).  The g gate (tanh path, ~4x the sensitivity)
and the output projection stay fp32r.  W_i/W_o are pre-scaled by 64 on the
host so their sigma~0.02 values land in e4m3's normal range; the 1/64
dequant rides the activation's scale operand for free.  Measured end-to-end
error: ~1.4e-2.

x ships as bf16 (half the bytes on the critical ramp) and is cast to fp32r
by the otherwise-idle vector engine; the e4m3 copy of x for the i/o gates
is quantized host-side and ships as its own 1 MB DMA.  Six PSUM banks
accumulate (i,g,o) x (bc0,bc1) concurrently in phase 1; phase 2 draws its
accumulators from the same 6-bank pool (3-deep nt pipeline) and w2[0..1]
prefetch on the sync ring's phase-1 slack.
"""
from contextlib import ExitStack

import ml_dtypes
import numpy as np

import concourse.bass as bass
import concourse.tile as tile
from concourse import bacc, mybir
from concourse.bass_utils import run_bass_kernel_spmd

NCORES = 8
B, D, H = 8192, 1024, 2048
BL = B // NCORES          # 1024 batch rows per core
P = 128
NKD = D // P              # 8  k-tiles over input dim
NT = H // P               # 16 hh row-blocks (one i/g/o triple each)
NK2 = H // P              # 16 k-tiles over hidden dim
NBC = BL // 512           # 2  batch chunks of 512
SW = 64.0                 # host pre-scale on W_i / W_o before e4m3 quant
WARMUP_MMS = 9          # ends ~when the first x cast lands; 8 left a
                        # 0.46us PE idle (and a p-state ramp reset) at t=0
BF16 = mybir.dt.bfloat16
F8 = mybir.dt.float8e4
F32R = mybir.dt.float32r
F32 = mybir.dt.float32
E4NP = ml_dtypes.float8_e4m3
DR = mybir.MatmulPerfMode.DoubleRow
ACT_SIG = mybir.ActivationFunctionType.Sigmoid
ACT_TANH = mybir.ActivationFunctionType.Tanh
ACT_COPY = mybir.ActivationFunctionType.Copy


def _round_fp32r(a: np.ndarray) -> np.ndarray:
    """RNE-round fp32 to the fp32r grid (11 mantissa bits, low 12 bits 0)."""
    b = np.ascontiguousarray(a, dtype=np.float32).view(np.uint32)
    r = (b + np.uint32(0x7FF) + ((b >> np.uint32(12)) & np.uint32(1))) & np.uint32(
        0xFFFFF000
    )
    return r.view(np.float32)


def _build_program():
    nc = bacc.Bacc("TRN2", target_bir_lowering=False, debug=False,
                   num_devices=NCORES)

    x_d = nc.dram_tensor("x", [P, NKD, BL], BF16, kind="ExternalInput").ap()
    w1g_d = nc.dram_tensor("w1g", [NT, P, NKD * P], F32R,
                           kind="ExternalInput").ap()
    w1io_d = nc.dram_tensor("w1io", [NT, P, 2 * NKD * P], F8,
                            kind="ExternalInput").ap()
    w2_d = nc.dram_tensor("w2", [8, P, NK2 * P], F32R,
                          kind="ExternalInput").ap()
    bias_d = nc.dram_tensor("bias", [P, 3 * NT], F32, kind="ExternalInput").ap()
    out_d = nc.dram_tensor("outT", [D, BL], F32, kind="ExternalOutput").ap()

    with tile.TileContext(nc) as tc, ExitStack() as ctx:
        xpool = ctx.enter_context(tc.tile_pool(name="xpool", bufs=1))
        hpool = ctx.enter_context(tc.tile_pool(name="hpool", bufs=1))
        bpool = ctx.enter_context(tc.tile_pool(name="bpool", bufs=1))
        w1gpool = ctx.enter_context(tc.tile_pool(name="w1gpool", bufs=3))
        w1iopool = ctx.enter_context(tc.tile_pool(name="w1iopool", bufs=3))
        w2pool = ctx.enter_context(tc.tile_pool(name="w2pool", bufs=3))
        apool = ctx.enter_context(tc.tile_pool(name="apool", bufs=2))
        opool = ctx.enter_context(tc.tile_pool(name="opool", bufs=2))
        ps1 = ctx.enter_context(tc.tile_pool(name="ps1", bufs=6, space="PSUM"))

        # Critical path to the first matmul: w1g[t=0] k-chunks on the sync
        # HWDGE ring; x k-tiles arrive as bf16 (half the bytes) on the
        # scalar/gpsimd rings and are cast to fp32r by the (idle) vector
        # engine, double-buffered through 4 staging tiles.
        w1g_sb0 = w1gpool.tile([P, NKD * P], F32R, tag="w1g", name="w1g_0")
        nc.sync.dma_start(w1g_sb0[:, 0:128], w1g_d[0][:, 0:128])
        x_sb = [xpool.tile([P, BL], F32R, tag=f"x{k}", name=f"x{k}")
                for k in range(NKD)]
        x_stg = [xpool.tile([P, BL], BF16, tag=f"xs{k}", name=f"xs{k}")
                 for k in range(NKD)]
        xh_sb = xpool.tile([P, NKD, BL], F8, tag="xh")
        # high_priority pins the x staging DMAs (and casts) to the front of
        # the schedule: DMA completion sems share 8 round-robin lanes in
        # schedule order, and a cast whose lane sits behind a 1.5 MB w1
        # tile load stalls the whole ramp by ~7us (measured).
        with tc.high_priority():
            for k in range(NKD):
                eng = nc.gpsimd if k % 2 else nc.scalar
                eng.dma_start(x_stg[k][:], x_d[:, k, :])
                nc.vector.tensor_copy(x_sb[k][:], x_stg[k][:])
        for k in range(1, NKD):
            nc.sync.dma_start(w1g_sb0[:, k * 128:(k + 1) * 128],
                              w1g_d[0][:, k * 128:(k + 1) * 128])
        # xh (the e4m3 copy of x for the i/o gates) is derived on-device by
        # the scalar engine, idle until t=0's activations: one Copy per
        # k-tile, straight from the bf16 staging tile so it depends only on
        # that tile's DMA, not on the vector cast (fp32r(bf16) == bf16, so
        # the result is bit-identical either way).  Eight distinct staging
        # tiles keep these reads from blocking later x arrivals.  The three
        # DMA queues (~110 GB/s each) are saturated for the first ~20 us,
        # so shipping xh from HBM would cost 1 MB of that budget.
        for k in range(NKD):
            nc.scalar.activation(xh_sb[:, k, :], x_stg[k][:], ACT_COPY)
        bias_sb = bpool.tile([P, 3 * NT], F32)
        nc.gpsimd.dma_start(bias_sb[:], bias_d[:])
        h_sb = hpool.tile([P, NK2 * BL], F32R)          # [hh_p, k2*BL + b]

        if WARMUP_MMS:
            # Warm the PE clock (HAM un-throttles after ~3.4us of activity)
            # while the prologue DMAs are still in flight.
            warm_sb = bpool.tile([P, 512], BF16)
            nc.vector.memset(warm_sb[:], 0.0)
            warm_ps = ps1.tile([P, 512], F32, tag="ps1")
            for _ in range(WARMUP_MMS):
                nc.tensor.matmul(warm_ps[:], warm_sb[:, 0:P], warm_sb[:],
                                 start=True, stop=True)

        # ---- phase 1: gates + activations -> h ----
        # Bank map: (gate, bc) with gate 0=i, 1=g, 2=o.
        w2_tiles = {}
        for t in range(NT):
            if t == 0:
                w1g_sb = w1g_sb0
                w1io_sb = w1iopool.tile([P, 2 * NKD, P], F8, tag="w1io")
                nc.sync.dma_start(w1io_sb[:], w1io_d[0])
            else:
                w1g_sb = w1gpool.tile([P, NKD * P], F32R, tag="w1g")
                w1io_sb = w1iopool.tile([P, 2 * NKD, P], F8, tag="w1io")
                if t <= 2:
                    # quarter-chunks: whatever shares a completion-sem lane
                    # with these only ever waits on a short transfer
                    for q in range(4):
                        nc.sync.dma_start(w1g_sb[:, q * 256:(q + 1) * 256],
                                          w1g_d[t][:, q * 256:(q + 1) * 256])
                else:
                    nc.sync.dma_start(w1g_sb[:], w1g_d[t])
                # from t=3 the prologue burst is over; w1io moves to the
                # gpsimd queue (empty until the phase-2 out stores, and its
                # triggers fire instantly) so sync carries only 512 KB per
                # 7.1 us t-iteration instead of 768 KB — removing the
                # ~53 ns every-other-t hiccups seen at 768 KB/t.
                eng = nc.gpsimd if t >= 3 else nc.sync
                eng.dma_start(w1io_sb[:], w1io_d[t])
            if t == 10 or t == 13:
                # prefetch the first two phase-2 weight tiles on the sync
                # ring's phase-1 slack
                j = 0 if t == 10 else 1
                w2_tiles[j] = w2pool.tile([P, NK2 * P], F32R, tag="w2",
                                          name=f"w2_{j}")
                nc.sync.dma_start(w2_tiles[j][:], w2_d[j])

            ps = [ps1.tile([P, 512], F32, tag="ps1", name=f"ps1_{j}")
                  for j in range(6)]
            # g gate: 8 fp32r k-steps per batch chunk; each stationary tile
            # serves both batch halves back-to-back, so LDWEIGHTS hides.
            for k in range(NKD):
                for bc in range(NBC):
                    nc.tensor.matmul(
                        ps[1 * NBC + bc][:],
                        w1g_sb[:, k * P:(k + 1) * P],
                        x_sb[k][:, bc * 512:(bc + 1) * 512],
                        start=(k == 0), stop=(k == NKD - 1),
                    )
            # i,o gates: 4 fp8 DoubleRow k-steps (K=256 each) per batch
            # chunk; PSUM holds 64x the gate value (host pre-scale on W).
            for gi, bank in ((0, 0), (1, 2)):
                for kk in range(NKD // 2):
                    for bc in range(NBC):
                        nc.tensor.matmul(
                            ps[bank * NBC + bc][:],
                            w1io_sb[:, gi * NKD + 2 * kk:gi * NKD + 2 * kk + 2, :],
                            xh_sb[:, 2 * kk:2 * kk + 2, bc * 512:(bc + 1) * 512],
                            start=(kk == 0), stop=(kk == NKD // 2 - 1),
                            perf_mode=DR,
                        )
            for bc in range(NBC):
                gate_sb = []
                for gi, func, scale in ((0, ACT_SIG, 1.0 / SW),
                                        (1, ACT_TANH, 1.0),
                                        (2, ACT_SIG, 1.0 / SW)):
                    bias_ap = bias_sb[:, 3 * t + gi:3 * t + gi + 1]
                    g_sb = apool.tile([P, 512], F32, tag=f"act{gi}")
                    nc.scalar.activation(g_sb[:], ps[gi * NBC + bc][:], func,
                                         bias=bias_ap, scale=scale)
                    gate_sb.append(g_sb)
                c_sb = apool.tile([P, 512], F32, tag="c")
                nc.vector.tensor_mul(c_sb[:], gate_sb[0][:], gate_sb[1][:])
                tc_sb = apool.tile([P, 512], F32, tag="tanh_c")
                nc.scalar.activation(tc_sb[:], c_sb[:], ACT_TANH)
                h_slice = h_sb[:, t * BL + bc * 512:t * BL + (bc + 1) * 512]
                nc.vector.tensor_mul(h_slice, gate_sb[2][:], tc_sb[:])

        # ---- phase 2: outT = W_outT.T @ h ----
        # PSUM tiles come from the same 6-bank pool as phase 1, giving a
        # 3-deep nt pipeline so the PSUM->SBUF copy latency never gates the
        # next accumulation group.  PSUM->SBUF copies run on the idle
        # vector engine; out stores go on the gpsimd/scalar rings and never
        # block a load.  The last tile (nt=7) splits its stores across
        # rings to shorten the end-of-kernel critical chain.
        for nt in range(8):
            w2_sb = w2_tiles.pop(nt)
            if nt + 2 < 8:
                w2_tiles[nt + 2] = w2pool.tile([P, NK2 * P], F32R, tag="w2",
                                               name=f"w2_{nt + 2}")
                nc.sync.dma_start(w2_tiles[nt + 2][:], w2_d[nt + 2])
            if nt < 7:
                ps = [ps1.tile([P, 512], F32, tag="ps1", name=f"p2_{bc}")
                      for bc in range(NBC)]
                for k2 in range(NK2):
                    for bc in range(NBC):
                        nc.tensor.matmul(
                            ps[bc][:],
                            w2_sb[:, k2 * P:(k2 + 1) * P],
                            h_sb[:, k2 * BL + bc * 512:
                                 k2 * BL + (bc + 1) * 512],
                            start=(k2 == 0), stop=(k2 == NK2 - 1),
                        )
                for bc in range(NBC):
                    o_sb = opool.tile([P, 512], F32, tag=f"osb{bc}")
                    nc.vector.tensor_copy(o_sb[:], ps[bc][:])
                    eng = nc.gpsimd if bc == 0 else nc.scalar
                    eng.dma_start(
                        out_d[nt * P:(nt + 1) * P, bc * 512:(bc + 1) * 512],
                        o_sb[:],
                    )
            else:
                # Last tile: quarter-granularity drain.  Each 128-col slice
                # is copied and stored as soon as it exists, spread over
                # three DMA engines (sync is idle by now), so the exposed
                # end-of-kernel chain is one short copy + one 64 KB store
                # + queue drain instead of a full-tile drain.
                engs = (nc.gpsimd, nc.scalar, nc.sync, nc.gpsimd)
                for bc in range(NBC):
                    psl = ps1.tile([P, 512], F32, tag="ps1", name="p2_l")
                    for k2 in range(NK2):
                        nc.tensor.matmul(
                            psl[:],
                            w2_sb[:, k2 * P:(k2 + 1) * P],
                            h_sb[:, k2 * BL + bc * 512:
                                 k2 * BL + (bc + 1) * 512],
                            start=(k2 == 0), stop=(k2 == NK2 - 1),
                        )
                    o_sb = opool.tile([P, 512], F32, tag=f"osb{bc}")
                    for q in range(4):
                        sl = slice(q * 128, (q + 1) * 128)
                        nc.vector.tensor_copy(o_sb[:, sl], psl[:, sl])
                        engs[q].dma_start(
                            out_d[nt * P:(nt + 1) * P,
                                  bc * 512 + q * 128:
                                  bc * 512 + (q + 1) * 128],
                            o_sb[:, sl],
                        )

    nc.compile()
    return nc


_CACHE: dict = {}


def _get_program():
    if "nc" not in _CACHE:
        _CACHE["nc"] = _build_program()
    return _CACHE["nc"]


def _prep_inputs(x, W_ih, b_ih, b_hh, W_out):
    """Host-side reshape/quantize. Returns per-core input maps."""
    # gate rows: torch order i, f, g, o; f unused.
    Wi = W_ih[0:H]                                          # [2048, 1024]
    Wg = W_ih[2 * H:3 * H]
    Wo = W_ih[3 * H:4 * H]

    # w1g[t, p_d, k*128 + m] = Wg[t*128+m, k*128+p_d]  (fp32r)
    w1g = _round_fp32r(
        Wg.reshape(NT, P, NKD, P).transpose(0, 3, 2, 1).reshape(NT, P, NKD * P)
    )
    # w1io[t, p_d, (gi*8 + k)*128 + m] = 64*W_{i,o}[t*128+m, k*128+p_d] (e4m3)
    wio = np.stack([Wi, Wo], axis=0) * SW                   # [2, 2048, 1024]
    w1io = np.ascontiguousarray(
        wio.reshape(2, NT, P, NKD, P).transpose(1, 4, 0, 3, 2)
        .reshape(NT, P, 2 * NKD * P)
    ).astype(E4NP)

    bsum = (b_ih + b_hh).astype(np.float32)
    bias_sel = np.stack([bsum[0:H], bsum[2 * H:3 * H], bsum[3 * H:4 * H]],
                        axis=0)                             # [3, 2048] i,g,o
    # bias[p, 3*t + gi] = bias_sel[gi, t*128+p]
    bias = np.ascontiguousarray(
        bias_sel.reshape(3, NT, P).transpose(2, 1, 0).reshape(P, 3 * NT)
    )

    # w2[n_tile, p_hh, k2*128 + m] = W_out[n_tile*128+m, k2*128+p_hh]
    w2 = W_out[:, :H].reshape(8, P, NK2, P).transpose(0, 3, 2, 1) \
        .reshape(8, P, NK2 * P)
    w2 = _round_fp32r(w2)

    in_maps = []
    for c in range(NCORES):
        xc = x[c * BL:(c + 1) * BL]                         # [1024 b, 1024 d]
        # x_dev[p_d, k, b] = xc[b, k*128 + p_d]
        xT = np.ascontiguousarray(xc.reshape(BL, NKD, P).transpose(2, 1, 0))
        in_maps.append({
            "x": xT.astype(ml_dtypes.bfloat16),
            "w1g": w1g, "w1io": w1io, "w2": w2, "bias": bias,
        })
    return in_maps


def kernel(x, W_ih, b_ih, b_hh, W_read, b_read, W_out, b_out, **_ignored):
    x = np.asarray(x, dtype=np.float32)
    W_ih = np.asarray(W_ih, dtype=np.float32)
    b_ih = np.asarray(b_ih, dtype=np.float32)
    b_hh = np.asarray(b_hh, dtype=np.float32)
    W_out = np.asarray(W_out, dtype=np.float32)
    b_out = np.asarray(b_out, dtype=np.float32)

    nc = _get_program()
    in_maps = _prep_inputs(x, W_ih, b_ih, b_hh, W_out)
    res = run_bass_kernel_spmd(nc, in_maps, list(range(NCORES)))

    out = np.empty((B, D), dtype=np.float32)
    for c in range(NCORES):
        out[c * BL:(c + 1) * BL] = res.results[c]["outT"].T
    out += b_out[None, :]
    return out


# revision 38
# speedup vs baseline: 1.0116x; 1.0116x over previous
"""NTM forward kernel for 8 Trainium2 NeuronCores (data-parallel over batch).

The reference NTM runs a single step from zero state: memory == 0, so the
read vector is exactly zero and the whole addressing path drops out.  What
remains is:
    gates = x @ W_ih.T + (b_ih + b_hh)        (f-gate unused: c0 == 0)
    h     = sigmoid(o) * tanh(sigmoid(i) * tanh(g))
    out   = h @ W_out[:, :H].T + b_out

Precision strategy (tolerance is 2e-2 L2-relative; fp32r everywhere
measures 1.8e-3): the i and o gates sit inside sigmoids whose outputs are
then attenuated through the h product, so ~3.7% gate error contributes
only ~1.4% to the output.  They run as fp8-e4m3 DoubleRow matmuls (K=256
per instruction, 2x fp32r throughput, 215 ns vs 229 ns per instruction
measured for half the work).  The g gate (tanh path, ~4x the sensitivity)
and the output projection stay fp32r.  W_i/W_o are pre-scaled by 64 on the
host so their sigma~0.02 values land in e4m3's normal range; the 1/64
dequant rides the activation's scale operand for free.  Measured end-to-end
error: ~1.4e-2.

x ships as bf16 (half the bytes on the critical ramp) and is cast to fp32r
by the otherwise-idle vector engine; the e4m3 copy of x for the i/o gates
is quantized host-side and ships as its own 1 MB DMA.  Six PSUM banks
accumulate (i,g,o) x (bc0,bc1) concurrently in phase 1; phase 2 draws its
accumulators from the same 6-bank pool (3-deep nt pipeline) and w2[0..1]
prefetch on the sync ring's phase-1 slack.
"""
from contextlib import ExitStack

import ml_dtypes
import numpy as np

import concourse.bass as bass
import concourse.tile as tile
from concourse import bacc, mybir
from concourse.bass_utils import run_bass_kernel_spmd

NCORES = 8
B, D, H = 8192, 1024, 2048
BL = B // NCORES          # 1024 batch rows per core
P = 128
NKD = D // P              # 8  k-tiles over input dim
NT = H // P               # 16 hh row-blocks (one i/g/o triple each)
NK2 = H // P              # 16 k-tiles over hidden dim
NBC = BL // 512           # 2  batch chunks of 512
SW = 64.0                 # host pre-scale on W_i / W_o before e4m3 quant
WARMUP_MMS = 8
BF16 = mybir.dt.bfloat16
F8 = mybir.dt.float8e4
F32R = mybir.dt.float32r
F32 = mybir.dt.float32
E4NP = ml_dtypes.float8_e4m3
DR = mybir.MatmulPerfMode.DoubleRow
ACT_SIG = mybir.ActivationFunctionType.Sigmoid
ACT_TANH = mybir.ActivationFunctionType.Tanh
ACT_COPY = mybir.ActivationFunctionType.Copy


def _round_fp32r(a: np.ndarray) -> np.ndarray:
    """RNE-round fp32 to the fp32r grid (11 mantissa bits, low 12 bits 0)."""
    b = np.ascontiguousarray(a, dtype=np.float32).view(np.uint32)
    r = (b + np.uint32(0x7FF) + ((b >> np.uint32(12)) & np.uint32(1))) & np.uint32(
        0xFFFFF000
    )
    return r.view(np.float32)


def _build_program():
    nc = bacc.Bacc("TRN2", target_bir_lowering=False, debug=False,
                   num_devices=NCORES)

    x_d = nc.dram_tensor("x", [P, NKD, BL], BF16, kind="ExternalInput").ap()
    w1g_d = nc.dram_tensor("w1g", [NT, P, NKD * P], F32R,
                           kind="ExternalInput").ap()
    w1io_d = nc.dram_tensor("w1io", [NT, P, 2 * NKD * P], F8,
                            kind="ExternalInput").ap()
    w2_d = nc.dram_tensor("w2", [8, P, NK2 * P], F32R,
                          kind="ExternalInput").ap()
    bias_d = nc.dram_tensor("bias", [P, 3 * NT], F32, kind="ExternalInput").ap()
    out_d = nc.dram_tensor("outT", [D, BL], F32, kind="ExternalOutput").ap()

    with tile.TileContext(nc) as tc, ExitStack() as ctx:
        xpool = ctx.enter_context(tc.tile_pool(name="xpool", bufs=1))
        hpool = ctx.enter_context(tc.tile_pool(name="hpool", bufs=1))
        bpool = ctx.enter_context(tc.tile_pool(name="bpool", bufs=1))
        w1gpool = ctx.enter_context(tc.tile_pool(name="w1gpool", bufs=3))
        w1iopool = ctx.enter_context(tc.tile_pool(name="w1iopool", bufs=3))
        w2pool = ctx.enter_context(tc.tile_pool(name="w2pool", bufs=3))
        apool = ctx.enter_context(tc.tile_pool(name="apool", bufs=2))
        opool = ctx.enter_context(tc.tile_pool(name="opool", bufs=2))
        ps1 = ctx.enter_context(tc.tile_pool(name="ps1", bufs=6, space="PSUM"))

        # Critical path to the first matmul: w1g[t=0] k-chunks on the sync
        # HWDGE ring; x k-tiles arrive as bf16 (half the bytes) on the
        # scalar/gpsimd rings and are cast to fp32r by the (idle) vector
        # engine, double-buffered through 4 staging tiles.
        w1g_sb0 = w1gpool.tile([P, NKD * P], F32R, tag="w1g", name="w1g_0")
        nc.sync.dma_start(w1g_sb0[:, 0:128], w1g_d[0][:, 0:128])
        x_sb = [xpool.tile([P, BL], F32R, tag=f"x{k}", name=f"x{k}")
                for k in range(NKD)]
        x_stg = [xpool.tile([P, BL], BF16, tag=f"xs{k}", name=f"xs{k}")
                 for k in range(NKD)]
        xh_sb = xpool.tile([P, NKD, BL], F8, tag="xh")
        # high_priority pins the x staging DMAs (and casts) to the front of
        # the schedule: DMA completion sems share 8 round-robin lanes in
        # schedule order, and a cast whose lane sits behind a 1.5 MB w1
        # tile load stalls the whole ramp by ~7us (measured).
        with tc.high_priority():
            for k in range(NKD):
                eng = nc.gpsimd if k % 2 else nc.scalar
                eng.dma_start(x_stg[k][:], x_d[:, k, :])
                nc.vector.tensor_copy(x_sb[k][:], x_stg[k][:])
        for k in range(1, NKD):
            nc.sync.dma_start(w1g_sb0[:, k * 128:(k + 1) * 128],
                              w1g_d[0][:, k * 128:(k + 1) * 128])
        # xh (the e4m3 copy of x for the i/o gates) is derived on-device by
        # the scalar engine, idle until t=0's activations: one Copy per
        # k-tile, straight from the bf16 staging tile so it depends only on
        # that tile's DMA, not on the vector cast (fp32r(bf16) == bf16, so
        # the result is bit-identical either way).  Eight distinct staging
        # tiles keep these reads from blocking later x arrivals.  The three
        # DMA queues (~110 GB/s each) are saturated for the first ~20 us,
        # so shipping xh from HBM would cost 1 MB of that budget.
        for k in range(NKD):
            nc.scalar.activation(xh_sb[:, k, :], x_stg[k][:], ACT_COPY)
        bias_sb = bpool.tile([P, 3 * NT], F32)
        nc.gpsimd.dma_start(bias_sb[:], bias_d[:])
        h_sb = hpool.tile([P, NK2 * BL], F32R)          # [hh_p, k2*BL + b]

        if WARMUP_MMS:
            # Warm the PE clock (HAM un-throttles after ~3.4us of activity)
            # while the prologue DMAs are still in flight.
            warm_sb = bpool.tile([P, 512], BF16)
            nc.vector.memset(warm_sb[:], 0.0)
            warm_ps = ps1.tile([P, 512], F32, tag="ps1")
            for _ in range(WARMUP_MMS):
                nc.tensor.matmul(warm_ps[:], warm_sb[:, 0:P], warm_sb[:],
                                 start=True, stop=True)

        # ---- phase 1: gates + activations -> h ----
        # Bank map: (gate, bc) with gate 0=i, 1=g, 2=o.
        w2_tiles = {}
        for t in range(NT):
            if t == 0:
                w1g_sb = w1g_sb0
                w1io_sb = w1iopool.tile([P, 2 * NKD, P], F8, tag="w1io")
                nc.sync.dma_start(w1io_sb[:], w1io_d[0])
            else:
                w1g_sb = w1gpool.tile([P, NKD * P], F32R, tag="w1g")
                w1io_sb = w1iopool.tile([P, 2 * NKD, P], F8, tag="w1io")
                if t <= 2:
                    # quarter-chunks: whatever shares a completion-sem lane
                    # with these only ever waits on a short transfer
                    for q in range(4):
                        nc.sync.dma_start(w1g_sb[:, q * 256:(q + 1) * 256],
                                          w1g_d[t][:, q * 256:(q + 1) * 256])
                else:
                    nc.sync.dma_start(w1g_sb[:], w1g_d[t])
                # from t=3 the prologue burst is over; w1io moves to the
                # gpsimd queue (empty until the phase-2 out stores, and its
                # triggers fire instantly) so sync carries only 512 KB per
                # 7.1 us t-iteration instead of 768 KB — removing the
                # ~53 ns every-other-t hiccups seen at 768 KB/t.
                eng = nc.gpsimd if t >= 3 else nc.sync
                eng.dma_start(w1io_sb[:], w1io_d[t])
            if t == 10 or t == 13:
                # prefetch the first two phase-2 weight tiles on the sync
                # ring's phase-1 slack
                j = 0 if t == 10 else 1
                w2_tiles[j] = w2pool.tile([P, NK2 * P], F32R, tag="w2",
                                          name=f"w2_{j}")
                nc.sync.dma_start(w2_tiles[j][:], w2_d[j])

            ps = [ps1.tile([P, 512], F32, tag="ps1", name=f"ps1_{j}")
                  for j in range(6)]
            # g gate: 8 fp32r k-steps per batch chunk; each stationary tile
            # serves both batch halves back-to-back, so LDWEIGHTS hides.
            for k in range(NKD):
                for bc in range(NBC):
                    nc.tensor.matmul(
                        ps[1 * NBC + bc][:],
                        w1g_sb[:, k * P:(k + 1) * P],
                        x_sb[k][:, bc * 512:(bc + 1) * 512],
                        start=(k == 0), stop=(k == NKD - 1),
                    )
            # i,o gates: 4 fp8 DoubleRow k-steps (K=256 each) per batch
            # chunk; PSUM holds 64x the gate value (host pre-scale on W).
            for gi, bank in ((0, 0), (1, 2)):
                for kk in range(NKD // 2):
                    for bc in range(NBC):
                        nc.tensor.matmul(
                            ps[bank * NBC + bc][:],
                            w1io_sb[:, gi * NKD + 2 * kk:gi * NKD + 2 * kk + 2, :],
                            xh_sb[:, 2 * kk:2 * kk + 2, bc * 512:(bc + 1) * 512],
                            start=(kk == 0), stop=(kk == NKD // 2 - 1),
                            perf_mode=DR,
                        )
            for bc in range(NBC):
                gate_sb = []
                for gi, func, scale in ((0, ACT_SIG, 1.0 / SW),
                                        (1, ACT_TANH, 1.0),
                                        (2, ACT_SIG, 1.0 / SW)):
                    bias_ap = bias_sb[:, 3 * t + gi:3 * t + gi + 1]
                    g_sb = apool.tile([P, 512], F32, tag=f"act{gi}")
                    nc.scalar.activation(g_sb[:], ps[gi * NBC + bc][:], func,
                                         bias=bias_ap, scale=scale)
                    gate_sb.append(g_sb)
                c_sb = apool.tile([P, 512], F32, tag="c")
                nc.vector.tensor_mul(c_sb[:], gate_sb[0][:], gate_sb[1][:])
                tc_sb = apool.tile([P, 512], F32, tag="tanh_c")
                nc.scalar.activation(tc_sb[:], c_sb[:], ACT_TANH)
                h_slice = h_sb[:, t * BL + bc * 512:t * BL + (bc + 1) * 512]
                nc.vector.tensor_mul(h_slice, gate_sb[2][:], tc_sb[:])

        # ---- phase 2: outT = W_outT.T @ h ----
        # PSUM tiles come from the same 6-bank pool as phase 1, giving a
        # 3-deep nt pipeline so the PSUM->SBUF copy latency never gates the
        # next accumulation group.  PSUM->SBUF copies run on the idle
        # vector engine; out stores go on the gpsimd/scalar rings and never
        # block a load.  The last tile (nt=7) splits its stores across
        # rings to shorten the end-of-kernel critical chain.
        for nt in range(8):
            w2_sb = w2_tiles.pop(nt)
            if nt + 2 < 8:
                w2_tiles[nt + 2] = w2pool.tile([P, NK2 * P], F32R, tag="w2",
                                               name=f"w2_{nt + 2}")
                nc.sync.dma_start(w2_tiles[nt + 2][:], w2_d[nt + 2])
            if nt < 7:
                ps = [ps1.tile([P, 512], F32, tag="ps1", name=f"p2_{bc}")
                      for bc in range(NBC)]
                for k2 in range(NK2):
                    for bc in range(NBC):
                        nc.tensor.matmul(
                            ps[bc][:],
                            w2_sb[:, k2 * P:(k2 + 1) * P],
                            h_sb[:, k2 * BL + bc * 512:
                                 k2 * BL + (bc + 1) * 512],
                            start=(k2 == 0), stop=(k2 == NK2 - 1),
                        )
                for bc in range(NBC):
                    o_sb = opool.tile([P, 512], F32, tag=f"osb{bc}")
                    nc.vector.tensor_copy(o_sb[:], ps[bc][:])
                    eng = nc.gpsimd if bc == 0 else nc.scalar
                    eng.dma_start(
                        out_d[nt * P:(nt + 1) * P, bc * 512:(bc + 1) * 512],
                        o_sb[:],
                    )
            else:
                # Last tile: quarter-granularity drain.  Each 128-col slice
                # is copied and stored as soon as it exists, spread over
                # three DMA engines (sync is idle by now), so the exposed
                # end-of-kernel chain is one short copy + one 64 KB store
                # + queue drain instead of a full-tile drain.
                engs = (nc.gpsimd, nc.scalar, nc.sync, nc.gpsimd)
                for bc in range(NBC):
                    psl = ps1.tile([P, 512], F32, tag="ps1", name="p2_l")
                    for k2 in range(NK2):
                        nc.tensor.matmul(
                            psl[:],
                            w2_sb[:, k2 * P:(k2 + 1) * P],
                            h_sb[:, k2 * BL + bc * 512:
                                 k2 * BL + (bc + 1) * 512],
                            start=(k2 == 0), stop=(k2 == NK2 - 1),
                        )
                    o_sb = opool.tile([P, 512], F32, tag=f"osb{bc}")
                    for q in range(4):
                        sl = slice(q * 128, (q + 1) * 128)
                        nc.vector.tensor_copy(o_sb[:, sl], psl[:, sl])
                        engs[q].dma_start(
                            out_d[nt * P:(nt + 1) * P,
                                  bc * 512 + q * 128:
                                  bc * 512 + (q + 1) * 128],
                            o_sb[:, sl],
                        )

    nc.compile()
    return nc


_CACHE: dict = {}


def _get_program():
    if "nc" not in _CACHE:
        _CACHE["nc"] = _build_program()
    return _CACHE["nc"]


def _prep_inputs(x, W_ih, b_ih, b_hh, W_out):
    """Host-side reshape/quantize. Returns per-core input maps."""
    # gate rows: torch order i, f, g, o; f unused.
    Wi = W_ih[0:H]                                          # [2048, 1024]
    Wg = W_ih[2 * H:3 * H]
    Wo = W_ih[3 * H:4 * H]

    # w1g[t, p_d, k*128 + m] = Wg[t*128+m, k*128+p_d]  (fp32r)
    w1g = _round_fp32r(
        Wg.reshape(NT, P, NKD, P).transpose(0, 3, 2, 1).reshape(NT, P, NKD * P)
    )
    # w1io[t, p_d, (gi*8 + k)*128 + m] = 64*W_{i,o}[t*128+m, k*128+p_d] (e4m3)
    wio = np.stack([Wi, Wo], axis=0) * SW                   # [2, 2048, 1024]
    w1io = np.ascontiguousarray(
        wio.reshape(2, NT, P, NKD, P).transpose(1, 4, 0, 3, 2)
        .reshape(NT, P, 2 * NKD * P)
    ).astype(E4NP)

    bsum = (b_ih + b_hh).astype(np.float32)
    bias_sel = np.stack([bsum[0:H], bsum[2 * H:3 * H], bsum[3 * H:4 * H]],
                        axis=0)                             # [3, 2048] i,g,o
    # bias[p, 3*t + gi] = bias_sel[gi, t*128+p]
    bias = np.ascontiguousarray(
        bias_sel.reshape(3, NT, P).transpose(2, 1, 0).reshape(P, 3 * NT)
    )

    # w2[n_tile, p_hh, k2*128 + m] = W_out[n_tile*128+m, k2*128+p_hh]
    w2 = W_out[:, :H].reshape(8, P, NK2, P).transpose(0, 3, 2, 1) \
        .reshape(8, P, NK2 * P)
    w2 = _round_fp32r(w2)

    in_maps = []
    for c in range(NCORES):
        xc = x[c * BL:(c + 1) * BL]                         # [1024 b, 1024 d]
        # x_dev[p_d, k, b] = xc[b, k*128 + p_d]
        xT = np.ascontiguousarray(xc.reshape(BL, NKD, P).transpose(2, 1, 0))
        in_maps.append({
            "x": xT.astype(ml_dtypes.bfloat16),
            "w1g": w1g, "w1io": w1io, "w2": w2, "bias": bias,
        })
    return in_maps


def kernel(x, W_ih, b_ih, b_hh, W_read, b_read, W_out, b_out, **_ignored):
    x = np.asarray(x, dtype=np.float32)
    W_ih = np.asarray(W_ih, dtype=np.float32)
    b_ih = np.asarray(b_ih, dtype=np.float32)
    b_hh = np.asarray(b_hh, dtype=np.float32)
    W_out = np.asarray(W_out, dtype=np.float32)
    b_out = np.asarray(b_out, dtype=np.float32)

    nc = _get_program()
    in_maps = _prep_inputs(x, W_ih, b_ih, b_hh, W_out)
    res = run_bass_kernel_spmd(nc, in_maps, list(range(NCORES)))

    out = np.empty((B, D), dtype=np.float32)
    for c in range(NCORES):
        out[c * BL:(c + 1) * BL] = res.results[c]["outT"].T
    out += b_out[None, :]
    return out
